# revision 5
# baseline (speedup 1.0000x reference)
"""GAT (2-layer graph attention network) Trainium2 kernel, v2.

Contract: kernel(**inputs) takes the FULL inputs from setup_inputs() and
returns the full (32, 256, 512) float32 output. Internally shards the batch
across 8 NeuronCores (4 graphs per core), runs a Bass/Tile kernel per core
(all 4 graphs' per-head chains interleaved for engine overlap), and
concatenates the results.

Key structure (vs the f32r v1 baseline, 277us -> ~184us):
  - bf16 everywhere: weights, one-hot, attention probabilities, h. Halves
    HBM traffic, enables FWL fast weight loads and 2x DVE modes.
  - mask add runs on the PE: host supplies mnT (transposed -1000*(1-adj));
    identity-matmuls accumulate it onto the e2-broadcast PSUM, so
    z = e2[m] + mneg[n,m] is built entirely by the tensor engine.
  - leaky_relu + e1-add fused into ONE scalar-engine op:
    Prelu(z + bias=e1col, alpha=0.2). Adding the mask before the leaky is
    fine because masked entries just need exp() to underflow to 0.
  - e1 bias columns for all heads come from one tiny per-graph PE transpose
    of the (embed@w1).T@onehot skinny matmul (shared with the e2 rows).
  - softmax normalization (and layer-2's non-pad mask) folds into the
    p-transpose: the transpose is a regular matmul against diag(1/zsum)
    instead of identity, so p is never normalized explicitly.
  - ELU exactly as min(exp(x) - 1, relu(x)): Act exp (f32 to keep small
    values exact), DVE relu, DVE fused subtract-min; result stored bf16.
  - layer-1 output is produced o-major (Wh.T @ pT), which is the hT layout
    layer 2 needs for its stationary operand - no h transpose.
"""

import numpy as np
import ml_dtypes
from contextlib import ExitStack

import concourse.bass as bass
import concourse.tile as tile
from concourse import mybir, bacc
from concourse.bass_utils import run_bass_kernel_spmd

f32 = mybir.dt.float32
bf16 = mybir.dt.bfloat16
AF = mybir.ActivationFunctionType
AL = mybir.AluOpType
BF = ml_dtypes.bfloat16

B, N, VOCAB, F, O, H, OUT = 32, 256, 200, 300, 256, 8, 512
NCORES = 8
GPC = B // NCORES
NC = N // 128
VC = 2
KC2 = (H * O) // 128
ALPHA = 0.2
MASK_NEG = -1000.0


def _build_nc():
    nc = bacc.Bacc("TRN2", target_bir_lowering=False, debug=False,
                   num_devices=NCORES)

    oh_d = nc.dram_tensor("oh", [GPC, 128, VC, N], bf16, kind="ExternalInput").ap()
    mnT_d = nc.dram_tensor("mnT", [GPC, 128, NC, NC, 128], bf16,
                           kind="ExternalInput").ap()
    npm_d = nc.dram_tensor("npm", [GPC, 128, NC], f32, kind="ExternalInput").ap()
    ew_d = nc.dram_tensor("embw", [128, H, VC, O], bf16,
                          kind="ExternalInput").ap()
    e2w_d = nc.dram_tensor("e12w", [128, VC, 2 * H], bf16,
                           kind="ExternalInput").ap()
    wo_d = nc.dram_tensor("woaug", [128, KC2, 2, 258], bf16,
                          kind="ExternalInput").ap()
    nw_d = nc.dram_tensor("negw", [1, 2, 258], bf16, kind="ExternalInput").ap()
    idb_d = nc.dram_tensor("identbf", [128, 128], bf16, kind="ExternalInput").ap()
    on3_d = nc.dram_tensor("ones3", [65, 128], bf16, kind="ExternalInput").ap()
    out_d = nc.dram_tensor("out", [GPC, 128, NC, OUT], f32,
                           kind="ExternalOutput").ap()

    with tile.TileContext(nc) as tc, ExitStack() as ctx:
        const = ctx.enter_context(tc.tile_pool(name="const", bufs=1))
        gpool = ctx.enter_context(tc.tile_pool(name="gpool", bufs=4))
        hpool = ctx.enter_context(tc.tile_pool(name="hpool", bufs=4))
        whpool = ctx.enter_context(tc.tile_pool(name="whpool", bufs=8))
        hbig = ctx.enter_context(tc.tile_pool(name="hbig", bufs=4))
        ps_aug = ctx.enter_context(tc.tile_pool(name="ps_aug", bufs=2, space="PSUM"))
        ps_z = ctx.enter_context(tc.tile_pool(name="ps_z", bufs=2, space="PSUM"))
        ps_tr = ctx.enter_context(tc.tile_pool(name="ps_tr", bufs=2, space="PSUM"))
        ps_out = ctx.enter_context(tc.tile_pool(name="ps_out", bufs=2, space="PSUM"))

        # ---- resident constants ----
        # order matters: small/early-needed tensors first so graph-0 head-0
        # can start while the big layer-2 weights still stream.
        identbf = const.tile([128, 128], bf16)
        nc.sync.dma_start(identbf[:], idb_d)
        warm = const.tile([1, 8], f32)
        nc.vector.memset(warm[:], 0.0)
        nc.scalar.activation(warm[:], warm[:], AF.Exp)
        ones3 = const.tile([65, 128], bf16)
        nc.sync.dma_start(ones3[:], on3_d)
        e2w_sb = const.tile([128, VC, 2 * H], bf16)
        nc.sync.dma_start(e2w_sb[:].rearrange("p a b -> p (a b)"),
                          e2w_d.rearrange("p a b -> p (a b)"))
        negw = const.tile([1, 2, 258], bf16)
        nc.sync.dma_start(negw[:].rearrange("p a b -> p (a b)"),
                          nw_d.rearrange("p a b -> p (a b)"))
        ew_sb = const.tile([128, H, VC, O], bf16)
        for h in range(H):
            nc.sync.dma_start(ew_sb[:, h].rearrange("p a b -> p (a b)"),
                              ew_d[:, h].rearrange("p a b -> p (a b)"))
        wo_sb = const.tile([128, KC2, 2, 258], bf16)

        def emit_wo_dma():
            for k in range(KC2):
                nc.sync.dma_start(wo_sb[:, k].rearrange("p a b -> p (a b)"),
                                  wo_d[:, k].rearrange("p a b -> p (a b)"))

        G = {}

        def emit_setup(g):
            s = G[g] = {}
            oh_sb = gpool.tile([128, VC, N], bf16)
            nc.sync.dma_start(oh_sb[:], oh_d[g])
            mnT = gpool.tile([128, NC, NC, 128], bf16)
            nc.sync.dma_start(mnT[:], mnT_d[g])
            npm_sb = gpool.tile([128, NC], f32)
            nc.sync.dma_start(npm_sb[:], npm_d[g])
            # e1/e2 rows for all heads: (16, N) = (embed@[w1|w2]).T @ onehot
            e2ps = ps_aug.tile([16, N], f32, tag="aug")
            for vc in range(VC):
                nc.tensor.matmul(e2ps[:], lhsT=e2w_sb[:, vc, :],
                                 rhs=oh_sb[:, vc, :],
                                 start=(vc == 0), stop=(vc == VC - 1))
            e12all = gpool.tile([16, N], bf16)
            nc.vector.tensor_copy(e12all[:], e2ps[:])
            # group e2 head rows onto base partitions 0/32/64 (3 per group)
            e2rs = gpool.tile([65, 3, N], bf16)
            for i in range(3):
                nh = min(3, H - 3 * i)
                nc.sync.dma_start(e2rs[32 * i:32 * i + 1, 0:nh, :],
                                    e12all[8 + 3 * i:8 + 3 * i + nh, :])
            # e1 rows -> per-chunk bias columns [128, NC, 8] via tiny transpose
            e1ps = ps_aug.tile([128, NC, 8], f32, tag="aug")
            for c in range(NC):
                nc.tensor.matmul(e1ps[:, c, :],
                                 lhsT=e12all[0:8, c * 128:(c + 1) * 128],
                                 rhs=identbf[0:8, 0:8],
                                 start=(c == 0), stop=(c == NC - 1),
                                 skip_group_check=True)
            e1cols = gpool.tile([128, NC, 8], f32)
            nc.vector.tensor_copy(e1cols[:].rearrange("p a b -> p (a b)"),
                                  e1ps[:].rearrange("p a b -> p (a b)"))
            hT = hbig.tile([128, KC2, N], bf16)
            s.update(oh_sb=oh_sb, npm=npm_sb, mnT=mnT, e2rs=e2rs,
                     e1cols=e1cols, hT=hT, wh={})

        def emit_aug(g, h):
            s = G[g]
            wh_sb = whpool.tile([128, NC, O], bf16, tag="wh_sb")
            s["wh"][h] = wh_sb
            aug = ps_aug.tile([128, NC, O], f32, tag="aug")
            for c in range(NC):
                for vc in range(VC):
                    nc.tensor.matmul(
                        aug[:, c, :],
                        lhsT=s["oh_sb"][:, vc, c * 128:(c + 1) * 128],
                        rhs=ew_sb[:, h, vc, :],
                        start=(c == 0 and vc == 0), stop=(vc == VC - 1),
                        skip_group_check=True)
            nc.vector.tensor_copy(wh_sb[:].rearrange("p a b -> p (a b)"),
                                  aug[:].rearrange("p a b -> p (a b)"))

        def emit_softmax(s, zps, e1_of_c, tag):
            """Common tail: z PSUM -> normalized-transposed pT (bf16)."""
            zt = hpool.tile([128, NC, N], f32, tag="zt" + tag)
            p_sb = hpool.tile([128, NC, N], bf16, tag="p" + tag)
            zsum = hpool.tile([128, NC], f32, tag="zs" + tag)
            zinv = hpool.tile([128, NC], f32, tag="zi" + tag)
            for c in range(NC):
                nc.scalar.activation(zt[:, c, :], zps[:, c, :], AF.Prelu,
                                     bias=e1_of_c(c), alpha=ALPHA)
            for c in range(NC):
                nc.scalar.activation(p_sb[:, c, :], zt[:, c, :], AF.Exp,
                                     accum_out=zsum[:, c:c + 1])
            nc.vector.reciprocal(zinv[:], zsum[:])
            return p_sb, zinv

        def emit_ptrans(s, p_sb, scale_col_of_c, tag, copy_eng):
            """diag-scaled transpose of p -> pT bf16 in SBUF."""
            diag = hpool.tile([128, NC, 128], bf16, tag="dg" + tag)
            for c in range(NC):
                nc.vector.tensor_scalar_mul(diag[:, c, :], identbf[:],
                                            scale_col_of_c(c))
            tp4 = ps_tr.tile([128, NC * NC, 128], f32, tag="tr")
            for c in range(NC):
                for d in range(NC):
                    nc.tensor.matmul(
                        tp4[:, c * NC + d, :],
                        lhsT=p_sb[:, c, d * 128:(d + 1) * 128],
                        rhs=diag[:, c, :], start=True, stop=True)
            pT = hpool.tile([128, NC, N], bf16, tag="pT" + tag)
            if copy_eng == "scalar":
                nc.scalar.copy(
                    pT[:].rearrange("p d (c u) -> p c d u", u=128),
                    tp4[:].rearrange("p (c d) u -> p c d u", d=NC))
            else:
                nc.vector.tensor_copy(
                    pT[:].rearrange("p d (c u) -> p c d u", u=128),
                    tp4[:].rearrange("p (c d) u -> p c d u", d=NC))
            return pT

        def emit_head(g, h):
            s = G[g]
            wh_sb = s["wh"].pop(h)
            mnT, hT = s["mnT"], s["hT"]
            # z = e2 broadcast + mneg, built on the PE
            zps = ps_z.tile([128, NC, N], f32, tag="z")
            gi, gj = h // 3, h % 3
            e2a = s["e2rs"][32 * gi:32 * gi + 1, gj, :]
            e2rep = bass.AP(tensor=e2a.tensor, offset=e2a.offset,
                            ap=[e2a.ap[0], [0, NC], [1, N]])
            nc.tensor.matmul(zps[:].rearrange("p a b -> p (a b)"),
                             lhsT=ones3[32 * gi:32 * gi + 1, :], rhs=e2rep,
                             start=True, stop=False, skip_group_check=True)
            for c in range(NC):
                for d in range(NC):
                    nc.tensor.matmul(
                        zps[:, c, d * 128:(d + 1) * 128],
                        lhsT=mnT[:, d, c, :], rhs=identbf[:],
                        start=False, stop=(c == NC - 1 and d == NC - 1),
                        skip_group_check=True)
            p_sb, zinv = emit_softmax(
                s, zps, lambda c: s["e1cols"][:, c, h:h + 1], "1")
            pT = emit_ptrans(s, p_sb, lambda c: zinv[:, c:c + 1], "1",
                             "scalar" if h % 3 == 0 else "vector")
            # out1T[o, n] = Wh.T @ pT (normalized); ELU -> hT rows
            ops = ps_out.tile([128, NC, N], f32, tag="big")
            for oc in range(NC):
                for mc in range(NC):
                    nc.tensor.matmul(
                        ops[:, oc, :], lhsT=wh_sb[:, mc, oc * 128:(oc + 1) * 128],
                        rhs=pT[:, mc, :], start=(mc == 0), stop=(mc == NC - 1))
            ex = hpool.tile([128, NC, N], f32, tag="ex")
            r1 = hpool.tile([128, NC, N], bf16, tag="r1")
            nc.scalar.activation(
                ex[:].rearrange("p a b -> p (a b)"),
                ops[:].rearrange("p a b -> p (a b)"), AF.Exp)
            nc.vector.tensor_scalar(
                r1[:].rearrange("p a b -> p (a b)"),
                ops[:].rearrange("p a b -> p (a b)"),
                0.0, None, op0=AL.max)
            # hT rows hold elu = min(exp(x) - 1, relu(x))
            nc.vector.scalar_tensor_tensor(
                hT[:, h * NC:(h + 1) * NC, :], ex[:], 1.0, r1[:],
                op0=AL.subtract, op1=AL.min)

        def emit_l2a(g):
            s = G[g]
            npm_sb, hT = s["npm"], s["hT"]
            wh2_sb = gpool.tile([128, NC, 2, 258], bf16)
            s["wh2_sb"] = wh2_sb
            for c in range(NC):
                for half in range(2):
                    hps = ps_aug.tile([128, 258], f32, tag="aug")
                    for k in range(KC2):
                        nc.tensor.matmul(
                            hps[:], lhsT=hT[:, k, c * 128:(c + 1) * 128],
                            rhs=wo_sb[:, k, half, :],
                            start=(k == 0), stop=(k == KC2 - 1))
                    nc.vector.tensor_scalar_mul(
                        wh2_sb[:, c, half, :], hps[:], npm_sb[:, c:c + 1])

        def emit_l2b(g):
            s = G[g]
            npm_sb, mnT = s["npm"], s["mnT"]
            wh2_sb = s["wh2_sb"]
            # e2 row via PE transpose of the two column chunks, then broadcast
            e2r_ps = ps_aug.tile([1, N], f32, tag="aug")
            for c in range(NC):
                nc.tensor.matmul(e2r_ps[:, c * 128:(c + 1) * 128],
                                 lhsT=wh2_sb[:, c, 1, 256:257], rhs=identbf[:],
                                 start=True, stop=True)
            e2row2 = gpool.tile([1, N], bf16)
            nc.vector.tensor_copy(e2row2[:], e2r_ps[:])
            zps = ps_z.tile([128, NC, N], f32, tag="z")
            e2a = e2row2[:]
            e2rep = bass.AP(tensor=e2a.tensor, offset=e2a.offset,
                            ap=[e2a.ap[0], [0, NC], [1, N]])
            nc.tensor.matmul(zps[:].rearrange("p a b -> p (a b)"),
                             lhsT=ones3[0:1, :], rhs=e2rep,
                             start=True, stop=False, skip_group_check=True)
            for c in range(NC):
                for d in range(NC):
                    nc.tensor.matmul(
                        zps[:, c, d * 128:(d + 1) * 128],
                        lhsT=mnT[:, d, c, :], rhs=identbf[:],
                        start=False, stop=(c == NC - 1 and d == NC - 1),
                        skip_group_check=True)
            p2, z2inv = emit_softmax(
                s, zps, lambda c: wh2_sb[:, c, 0, 256:257], "2")
            sc2 = gpool.tile([128, NC], f32)
            nc.vector.tensor_mul(sc2[:], z2inv[:], npm_sb[:])
            pT2 = emit_ptrans(s, p2, lambda c: sc2[:, c:c + 1], "2", "scalar")
            out_sb = gpool.tile([128, NC, OUT], f32)
            for c in range(NC):
                o2ps = ps_out.tile([128, OUT], f32, tag="big")
                for mc in range(NC):
                    nc.tensor.matmul(
                        o2ps[:], lhsT=pT2[:, mc, c * 128:(c + 1) * 128],
                        rhs=wh2_sb[:, mc, :, 0:256], start=(mc == 0),
                        stop=(mc == NC - 1))
                a2 = hpool.tile([128, OUT], f32, tag="a2")
                r2 = hpool.tile([128, OUT], bf16, tag="r2")
                nc.scalar.activation(a2[:], o2ps[:], AF.Exp)
                nc.vector.tensor_scalar(r2[:], o2ps[:], 0.0, None, op0=AL.max)
                nc.vector.scalar_tensor_tensor(
                    out_sb[:, c, :], a2[:], 1.0, r2[:],
                    op0=AL.subtract, op1=AL.min)
                nc.gpsimd.dma_start(out_d[g, :, c], out_sb[:, c, :])
            del G[g]

        for g in range(GPC):
            emit_setup(g)
        emit_wo_dma()
        for g in range(GPC):
            emit_aug(g, 0)
        for h in range(H):
            for g in range(GPC):
                if h + 1 < H:
                    emit_aug(g, h + 1)
                emit_head(g, h)
        emit_l2a(0)
        emit_l2a(1)
        emit_l2b(0)
        emit_l2a(2)
        emit_l2b(1)
        emit_l2a(3)
        emit_l2b(2)
        emit_l2b(3)

    nc.compile()
    return nc


_NC_CACHE = {}


def build_kernel():
    if "nc" not in _NC_CACHE:
        _NC_CACHE["nc"] = _build_nc()
    return _NC_CACHE["nc"]


def _host_prep(fea, adj, non_pad_mask, embed, W_heads, a_heads, W_out, a_out):
    W64 = W_heads.astype(np.float64)
    w1 = np.einsum("hfo,ho->hf", W64, a_heads[:, :O].astype(np.float64))
    w2 = np.einsum("hfo,ho->hf", W64, a_heads[:, O:].astype(np.float64))
    emb64 = np.zeros((VC * 128, F))
    emb64[:VOCAB] = embed.astype(np.float64)
    embw = np.einsum("vf,hfo->hvo", emb64, W64)                 # (H, 256, O)
    embw = np.ascontiguousarray(
        embw.reshape(H, VC, 128, O).transpose(2, 0, 1, 3)).astype(BF)
    e12 = emb64 @ np.concatenate([w1.T, w2.T], axis=1)          # (256, 16)
    e2w = np.ascontiguousarray(
        e12.reshape(VC, 128, 2 * H).transpose(1, 0, 2)).astype(BF)

    Wo64 = W_out.astype(np.float64)
    w1o = Wo64 @ a_out[:OUT].astype(np.float64)
    w2o = Wo64 @ a_out[OUT:].astype(np.float64)
    zcol = np.zeros((H * O, 1))
    woaug = np.concatenate(
        [Wo64[:, 0:256], w1o[:, None], zcol,
         Wo64[:, 256:512], w2o[:, None], zcol], axis=1)       # (2048, 516)
    # rank-1 correction: hT holds elu+1, so subtract colsum(woaug)
    negw = -woaug.sum(axis=0).reshape(1, 2, 258).astype(BF)
    woaug = np.ascontiguousarray(
        woaug.reshape(KC2, 128, 2, 258).transpose(1, 0, 2, 3)).astype(BF)

    vidx = np.arange(VC * 128).reshape(VC, 128)
    oh = (fea[:, None, None, :] == vidx[None, :, :, None])       # (B, VC, 128, N)
    oh = np.ascontiguousarray(oh.transpose(0, 2, 1, 3)).astype(BF)

    # mnT[g, p, d, c, q] = mneg[c*128+q, d*128+p] ; mneg = -1000*(1-adj)
    mneg = MASK_NEG * (1.0 - (adj > 0))                          # (B, N, N) n,m
    mnT = mneg.transpose(0, 2, 1).reshape(B, NC, 128, NC, 128)   # g, d, p, c, q
    mnT = np.ascontiguousarray(mnT.transpose(0, 2, 1, 3, 4)).astype(BF)
    npm = np.ascontiguousarray(
        non_pad_mask.reshape(B, NC, 128).transpose(0, 2, 1)).astype(np.float32)

    return oh, mnT, npm, embw, e2w, woaug, negw


def kernel(fea, adj, non_pad_mask, embed, W_heads, a_heads, W_out, a_out,
           _trace=False):
    oh, mnT, npm, embw, e2w, woaug, negw = _host_prep(
        fea, adj, non_pad_mask, embed, W_heads, a_heads, W_out, a_out)

    nc = build_kernel()
    identbf = np.eye(128, dtype=np.float32).astype(BF)
    ones3 = np.ones((65, 128), dtype=np.float32).astype(BF)
    in_maps = []
    for i in range(NCORES):
        sl = slice(i * GPC, (i + 1) * GPC)
        in_maps.append({
            "oh": oh[sl], "mnT": mnT[sl], "npm": npm[sl],
            "embw": embw, "e12w": e2w, "woaug": woaug, "negw": negw,
            "identbf": identbf, "ones3": ones3,
        })
    res = run_bass_kernel_spmd(nc, in_maps, core_ids=list(range(NCORES)),
                               trace=_trace)
    outs = []
    for i in range(NCORES):
        o = res.results[i]["out"]                   # (GPC, 128, NC, OUT)
        outs.append(o.transpose(0, 2, 1, 3).reshape(GPC, N, OUT))
    full = np.concatenate(outs, axis=0).astype(np.float32)
    if _trace:
        kernel.last_results = res
    return full


# revision 6
# speedup vs baseline: 1.1311x; 1.1311x over previous
"""GAT (2-layer graph attention network) Trainium2 kernel, v2.

Contract: kernel(**inputs) takes the FULL inputs from setup_inputs() and
returns the full (32, 256, 512) float32 output. Internally shards the batch
across 8 NeuronCores (4 graphs per core), runs a Bass/Tile kernel per core
(all 4 graphs' per-head chains interleaved for engine overlap), and
concatenates the results.

Key structure (vs the f32r v1 baseline, 277us -> ~184us):
  - bf16 everywhere: weights, one-hot, attention probabilities, h. Halves
    HBM traffic, enables FWL fast weight loads and 2x DVE modes.
  - mask add runs on the PE: host supplies mnT (transposed -1000*(1-adj));
    identity-matmuls accumulate it onto the e2-broadcast PSUM, so
    z = e2[m] + mneg[n,m] is built entirely by the tensor engine.
  - leaky_relu + e1-add fused into ONE scalar-engine op:
    Prelu(z + bias=e1col, alpha=0.2). Adding the mask before the leaky is
    fine because masked entries just need exp() to underflow to 0.
  - e1 bias columns for all heads come from one tiny per-graph PE transpose
    of the (embed@w1).T@onehot skinny matmul (shared with the e2 rows).
  - softmax normalization (and layer-2's non-pad mask) folds into the
    p-transpose: the transpose is a regular matmul against diag(1/zsum)
    instead of identity, so p is never normalized explicitly.
  - ELU exactly as min(exp(x) - 1, relu(x)): Act exp (f32 to keep small
    values exact), DVE relu, DVE fused subtract-min; result stored bf16.
  - layer-1 output is produced o-major (Wh.T @ pT), which is the hT layout
    layer 2 needs for its stationary operand - no h transpose.
"""

import numpy as np
import ml_dtypes
from contextlib import ExitStack

import concourse.bass as bass
import concourse.tile as tile
from concourse import mybir, bacc
from concourse.bass_utils import run_bass_kernel_spmd

f32 = mybir.dt.float32
bf16 = mybir.dt.bfloat16
AF = mybir.ActivationFunctionType
AL = mybir.AluOpType
BF = ml_dtypes.bfloat16

B, N, VOCAB, F, O, H, OUT = 32, 256, 200, 300, 256, 8, 512
NCORES = 8
GPC = B // NCORES
NC = N // 128
VC = 2
KC2 = (H * O) // 128
ALPHA = 0.2
MASK_NEG = -1000.0


def _build_nc():
    nc = bacc.Bacc("TRN2", target_bir_lowering=False, debug=False,
                   num_devices=NCORES)

    oh_d = nc.dram_tensor("oh", [GPC, 128, VC, N], bf16, kind="ExternalInput").ap()
    mnT_d = nc.dram_tensor("mnT", [GPC, 128, NC, NC, 128], bf16,
                           kind="ExternalInput").ap()
    npm_d = nc.dram_tensor("npm", [GPC, 128, NC], f32, kind="ExternalInput").ap()
    ew_d = nc.dram_tensor("embw", [128, H, VC, O], bf16,
                          kind="ExternalInput").ap()
    e2w_d = nc.dram_tensor("e12w", [128, VC, 2 * H], bf16,
                           kind="ExternalInput").ap()
    wo_d = nc.dram_tensor("woaug", [128, KC2, 2, 258], bf16,
                          kind="ExternalInput").ap()
    nw_d = nc.dram_tensor("negw", [1, 2, 258], bf16, kind="ExternalInput").ap()
    idb_d = nc.dram_tensor("identbf", [128, 128], bf16, kind="ExternalInput").ap()
    on3_d = nc.dram_tensor("ones3", [65, 128], bf16, kind="ExternalInput").ap()
    out_d = nc.dram_tensor("out", [GPC, 128, NC, OUT], f32,
                           kind="ExternalOutput").ap()

    with tile.TileContext(nc) as tc, ExitStack() as ctx:
        const = ctx.enter_context(tc.tile_pool(name="const", bufs=1))
        gpool = ctx.enter_context(tc.tile_pool(name="gpool", bufs=4))
        hpool = ctx.enter_context(tc.tile_pool(name="hpool", bufs=4))
        whpool = ctx.enter_context(tc.tile_pool(name="whpool", bufs=8))
        hbig = ctx.enter_context(tc.tile_pool(name="hbig", bufs=4))
        ps_aug = ctx.enter_context(tc.tile_pool(name="ps_aug", bufs=2, space="PSUM"))
        ps_z = ctx.enter_context(tc.tile_pool(name="ps_z", bufs=2, space="PSUM"))
        ps_tr = ctx.enter_context(tc.tile_pool(name="ps_tr", bufs=2, space="PSUM"))
        ps_out = ctx.enter_context(tc.tile_pool(name="ps_out", bufs=2, space="PSUM"))

        # ---- resident constants ----
        # order matters: small/early-needed tensors first so graph-0 head-0
        # can start while the big layer-2 weights still stream.
        identbf = const.tile([128, 128], bf16)
        nc.sync.dma_start(identbf[:], idb_d)
        warm = const.tile([1, 8], f32)
        nc.vector.memset(warm[:], 0.0)
        nc.scalar.activation(warm[:], warm[:], AF.Exp)
        ones3 = const.tile([65, 128], bf16)
        nc.sync.dma_start(ones3[:], on3_d)
        e2w_sb = const.tile([128, VC, 2 * H], bf16)
        nc.sync.dma_start(e2w_sb[:].rearrange("p a b -> p (a b)"),
                          e2w_d.rearrange("p a b -> p (a b)"))
        negw = const.tile([1, 2, 258], bf16)
        nc.sync.dma_start(negw[:].rearrange("p a b -> p (a b)"),
                          nw_d.rearrange("p a b -> p (a b)"))
        ew_sb = const.tile([128, H, VC, O], bf16)
        for h in range(H):
            nc.sync.dma_start(ew_sb[:, h].rearrange("p a b -> p (a b)"),
                              ew_d[:, h].rearrange("p a b -> p (a b)"))
        wo_sb = const.tile([128, KC2, 2, 258], bf16)

        def emit_wo_dma():
            for k in range(KC2):
                nc.sync.dma_start(wo_sb[:, k].rearrange("p a b -> p (a b)"),
                                  wo_d[:, k].rearrange("p a b -> p (a b)"))

        G = {}

        def emit_setup(g):
            s = G[g] = {}
            oh_sb = gpool.tile([128, VC, N], bf16)
            nc.sync.dma_start(oh_sb[:], oh_d[g])
            mnT = gpool.tile([128, NC, NC, 128], bf16)
            nc.sync.dma_start(mnT[:], mnT_d[g])
            npm_sb = gpool.tile([128, NC], f32)
            nc.sync.dma_start(npm_sb[:], npm_d[g])
            # e1/e2 rows for all heads: (16, N) = (embed@[w1|w2]).T @ onehot
            e2ps = ps_aug.tile([16, N], f32, tag="aug")
            for vc in range(VC):
                nc.tensor.matmul(e2ps[:], lhsT=e2w_sb[:, vc, :],
                                 rhs=oh_sb[:, vc, :],
                                 start=(vc == 0), stop=(vc == VC - 1))
            e12all = gpool.tile([16, N], bf16)
            nc.vector.tensor_copy(e12all[:], e2ps[:])
            # group e2 head rows onto base partitions 0/32/64 (3 per group)
            e2rs = gpool.tile([65, 3, N], bf16)
            for i in range(3):
                nh = min(3, H - 3 * i)
                nc.sync.dma_start(e2rs[32 * i:32 * i + 1, 0:nh, :],
                                    e12all[8 + 3 * i:8 + 3 * i + nh, :])
            # e1 rows -> per-chunk bias columns [128, NC, 8] via tiny transpose
            e1ps = ps_aug.tile([128, NC, 8], f32, tag="aug")
            for c in range(NC):
                nc.tensor.matmul(e1ps[:, c, :],
                                 lhsT=e12all[0:8, c * 128:(c + 1) * 128],
                                 rhs=identbf[0:8, 0:8],
                                 start=(c == 0), stop=(c == NC - 1),
                                 skip_group_check=True)
            e1cols = gpool.tile([128, NC, 8], f32)
            nc.vector.tensor_copy(e1cols[:].rearrange("p a b -> p (a b)"),
                                  e1ps[:].rearrange("p a b -> p (a b)"))
            hT = hbig.tile([128, KC2, N], bf16)
            s.update(oh_sb=oh_sb, npm=npm_sb, mnT=mnT, e2rs=e2rs,
                     e1cols=e1cols, hT=hT, wh={})

        def emit_aug(g, h):
            s = G[g]
            wh_sb = whpool.tile([128, NC, O], bf16, tag="wh_sb")
            s["wh"][h] = wh_sb
            aug = ps_aug.tile([128, NC, O], f32, tag="aug")
            for c in range(NC):
                for vc in range(VC):
                    nc.tensor.matmul(
                        aug[:, c, :],
                        lhsT=s["oh_sb"][:, vc, c * 128:(c + 1) * 128],
                        rhs=ew_sb[:, h, vc, :],
                        start=(c == 0 and vc == 0), stop=(vc == VC - 1),
                        skip_group_check=True)
            nc.vector.tensor_copy(wh_sb[:].rearrange("p a b -> p (a b)"),
                                  aug[:].rearrange("p a b -> p (a b)"))

        def emit_softmax(s, zps, e1_of_c, tag):
            """Common tail: z PSUM -> normalized-transposed pT (bf16)."""
            zt = hpool.tile([128, NC, N], f32, tag="zt" + tag)
            p_sb = hpool.tile([128, NC, N], bf16, tag="p" + tag)
            zsum = hpool.tile([128, NC], f32, tag="zs" + tag)
            zinv = hpool.tile([128, NC], f32, tag="zi" + tag)
            for c in range(NC):
                nc.scalar.activation(zt[:, c, :], zps[:, c, :], AF.Prelu,
                                     bias=e1_of_c(c), alpha=ALPHA)
            for c in range(NC):
                nc.scalar.activation(p_sb[:, c, :], zt[:, c, :], AF.Exp,
                                     accum_out=zsum[:, c:c + 1])
            nc.vector.reciprocal(zinv[:], zsum[:])
            return p_sb, zinv

        def emit_ptrans(s, p_sb, scale_col_of_c, tag, copy_eng):
            """diag-scaled transpose of p -> pT bf16 in SBUF."""
            diag = hpool.tile([128, NC, 128], bf16, tag="dg" + tag)
            for c in range(NC):
                nc.vector.tensor_scalar_mul(diag[:, c, :], identbf[:],
                                            scale_col_of_c(c))
            tp4 = ps_tr.tile([128, NC * NC, 128], f32, tag="tr")
            for c in range(NC):
                for d in range(NC):
                    nc.tensor.matmul(
                        tp4[:, c * NC + d, :],
                        lhsT=p_sb[:, c, d * 128:(d + 1) * 128],
                        rhs=diag[:, c, :], start=True, stop=True)
            pT = hpool.tile([128, NC, N], bf16, tag="pT" + tag)
            if copy_eng == "scalar":
                nc.scalar.copy(
                    pT[:].rearrange("p d (c u) -> p c d u", u=128),
                    tp4[:].rearrange("p (c d) u -> p c d u", d=NC))
            else:
                nc.vector.tensor_copy(
                    pT[:].rearrange("p d (c u) -> p c d u", u=128),
                    tp4[:].rearrange("p (c d) u -> p c d u", d=NC))
            return pT

        def emit_head(g, h):
            s = G[g]
            wh_sb = s["wh"].pop(h)
            mnT, hT = s["mnT"], s["hT"]
            # z = e2 broadcast + mneg, built on the PE
            zps = ps_z.tile([128, NC, N], f32, tag="z")
            gi, gj = h // 3, h % 3
            e2a = s["e2rs"][32 * gi:32 * gi + 1, gj, :]
            e2rep = bass.AP(tensor=e2a.tensor, offset=e2a.offset,
                            ap=[e2a.ap[0], [0, NC], [1, N]])
            nc.tensor.matmul(zps[:].rearrange("p a b -> p (a b)"),
                             lhsT=ones3[32 * gi:32 * gi + 1, :], rhs=e2rep,
                             start=True, stop=False, skip_group_check=True)
            for c in range(NC):
                for d in range(NC):
                    nc.tensor.matmul(
                        zps[:, c, d * 128:(d + 1) * 128],
                        lhsT=mnT[:, d, c, :], rhs=identbf[:],
                        start=False, stop=(c == NC - 1 and d == NC - 1),
                        skip_group_check=True)
            p_sb, zinv = emit_softmax(
                s, zps, lambda c: s["e1cols"][:, c, h:h + 1], "1")
            pT = emit_ptrans(s, p_sb, lambda c: zinv[:, c:c + 1], "1",
                             "scalar" if h % 3 == 0 else "vector")
            # out1T[o, n] = Wh.T @ pT (normalized); ELU -> hT rows
            ops = ps_out.tile([128, NC, N], f32, tag="big")
            for oc in range(NC):
                for mc in range(NC):
                    nc.tensor.matmul(
                        ops[:, oc, :], lhsT=wh_sb[:, mc, oc * 128:(oc + 1) * 128],
                        rhs=pT[:, mc, :], start=(mc == 0), stop=(mc == NC - 1))
            ex = hpool.tile([128, NC, N], f32, tag="ex")
            r1 = hpool.tile([128, NC, N], bf16, tag="r1")
            nc.scalar.activation(
                ex[:].rearrange("p a b -> p (a b)"),
                ops[:].rearrange("p a b -> p (a b)"), AF.Exp)
            nc.vector.tensor_scalar(
                r1[:].rearrange("p a b -> p (a b)"),
                ops[:].rearrange("p a b -> p (a b)"),
                0.0, None, op0=AL.max)
            # hT rows hold elu = min(exp(x) - 1, relu(x))
            nc.vector.scalar_tensor_tensor(
                hT[:, h * NC:(h + 1) * NC, :], ex[:], 1.0, r1[:],
                op0=AL.subtract, op1=AL.min)

        def emit_l2a(g):
            s = G[g]
            npm_sb, hT = s["npm"], s["hT"]
            wh2_sb = gpool.tile([128, NC, 2, 258], bf16)
            s["wh2_sb"] = wh2_sb
            for c in range(NC):
                for half in range(2):
                    hps = ps_aug.tile([128, 258], f32, tag="aug")
                    for k in range(KC2):
                        nc.tensor.matmul(
                            hps[:], lhsT=hT[:, k, c * 128:(c + 1) * 128],
                            rhs=wo_sb[:, k, half, :],
                            start=(k == 0), stop=(k == KC2 - 1))
                    nc.vector.tensor_scalar_mul(
                        wh2_sb[:, c, half, :], hps[:], npm_sb[:, c:c + 1])

        def emit_l2b(g):
            s = G[g]
            npm_sb, mnT = s["npm"], s["mnT"]
            wh2_sb = s["wh2_sb"]
            # e2 row via PE transpose of the two column chunks, then broadcast
            e2r_ps = ps_aug.tile([1, N], f32, tag="aug")
            for c in range(NC):
                nc.tensor.matmul(e2r_ps[:, c * 128:(c + 1) * 128],
                                 lhsT=wh2_sb[:, c, 1, 256:257], rhs=identbf[:],
                                 start=True, stop=True)
            e2row2 = gpool.tile([1, N], bf16)
            nc.vector.tensor_copy(e2row2[:], e2r_ps[:])
            zps = ps_z.tile([128, NC, N], f32, tag="z")
            e2a = e2row2[:]
            e2rep = bass.AP(tensor=e2a.tensor, offset=e2a.offset,
                            ap=[e2a.ap[0], [0, NC], [1, N]])
            nc.tensor.matmul(zps[:].rearrange("p a b -> p (a b)"),
                             lhsT=ones3[0:1, :], rhs=e2rep,
                             start=True, stop=False, skip_group_check=True)
            for c in range(NC):
                for d in range(NC):
                    nc.tensor.matmul(
                        zps[:, c, d * 128:(d + 1) * 128],
                        lhsT=mnT[:, d, c, :], rhs=identbf[:],
                        start=False, stop=(c == NC - 1 and d == NC - 1),
                        skip_group_check=True)
            p2, z2inv = emit_softmax(
                s, zps, lambda c: wh2_sb[:, c, 0, 256:257], "2")
            sc2 = gpool.tile([128, NC], f32)
            nc.vector.tensor_mul(sc2[:], z2inv[:], npm_sb[:])
            pT2 = emit_ptrans(s, p2, lambda c: sc2[:, c:c + 1], "2", "scalar")
            out_sb = gpool.tile([128, NC, OUT], f32)
            for c in range(NC):
                o2ps = ps_out.tile([128, OUT], f32, tag="big")
                for mc in range(NC):
                    nc.tensor.matmul(
                        o2ps[:], lhsT=pT2[:, mc, c * 128:(c + 1) * 128],
                        rhs=wh2_sb[:, mc, :, 0:256], start=(mc == 0),
                        stop=(mc == NC - 1))
                a2 = hpool.tile([128, OUT], f32, tag="a2")
                r2 = hpool.tile([128, OUT], bf16, tag="r2")
                nc.scalar.activation(a2[:], o2ps[:], AF.Exp)
                nc.vector.tensor_scalar(r2[:], o2ps[:], 0.0, None, op0=AL.max)
                nc.vector.scalar_tensor_tensor(
                    out_sb[:, c, :], a2[:], 1.0, r2[:],
                    op0=AL.subtract, op1=AL.min)
                nc.gpsimd.dma_start(out_d[g, :, c], out_sb[:, c, :])
            del G[g]

        for g in range(GPC):
            emit_setup(g)
        emit_wo_dma()
        for g in range(GPC):
            emit_aug(g, 0)
        # skewed head schedule: g0 finishes first so its layer-2 overlaps
        # the remaining graphs' heads; one graph's l2a/l2b per round after.
        counts = [[2, 2, 2, 2, 0, 0, 0],
                  [2, 2, 1, 1, 2, 0, 0],
                  [2, 1, 1, 1, 1, 2, 0],
                  [1, 1, 1, 1, 1, 1, 2]]
        prog = [0, 0, 0, 0]
        stage = [0, 0, 0, 0]          # 0=heads, 1=l2a done, 2=l2b done
        for r in range(7):
            for g in range(GPC):
                for _ in range(counts[g][r]):
                    h = prog[g]
                    if h + 1 < H:
                        emit_aug(g, h + 1)
                    emit_head(g, h)
                    prog[g] += 1
            for g in range(GPC):
                if stage[g] == 1:
                    emit_l2b(g)
                    stage[g] = 2
                elif stage[g] == 0 and prog[g] == H:
                    emit_l2a(g)
                    stage[g] = 1
        for g in range(GPC):
            if stage[g] == 1:
                emit_l2b(g)
                stage[g] = 2

    nc.compile()
    return nc


_NC_CACHE = {}


def build_kernel():
    if "nc" not in _NC_CACHE:
        _NC_CACHE["nc"] = _build_nc()
    return _NC_CACHE["nc"]


def _host_prep(fea, adj, non_pad_mask, embed, W_heads, a_heads, W_out, a_out):
    W64 = W_heads.astype(np.float64)
    w1 = np.einsum("hfo,ho->hf", W64, a_heads[:, :O].astype(np.float64))
    w2 = np.einsum("hfo,ho->hf", W64, a_heads[:, O:].astype(np.float64))
    emb64 = np.zeros((VC * 128, F))
    emb64[:VOCAB] = embed.astype(np.float64)
    embw = np.einsum("vf,hfo->hvo", emb64, W64)                 # (H, 256, O)
    embw = np.ascontiguousarray(
        embw.reshape(H, VC, 128, O).transpose(2, 0, 1, 3)).astype(BF)
    e12 = emb64 @ np.concatenate([w1.T, w2.T], axis=1)          # (256, 16)
    e2w = np.ascontiguousarray(
        e12.reshape(VC, 128, 2 * H).transpose(1, 0, 2)).astype(BF)

    Wo64 = W_out.astype(np.float64)
    w1o = Wo64 @ a_out[:OUT].astype(np.float64)
    w2o = Wo64 @ a_out[OUT:].astype(np.float64)
    zcol = np.zeros((H * O, 1))
    woaug = np.concatenate(
        [Wo64[:, 0:256], w1o[:, None], zcol,
         Wo64[:, 256:512], w2o[:, None], zcol], axis=1)       # (2048, 516)
    # rank-1 correction: hT holds elu+1, so subtract colsum(woaug)
    negw = -woaug.sum(axis=0).reshape(1, 2, 258).astype(BF)
    woaug = np.ascontiguousarray(
        woaug.reshape(KC2, 128, 2, 258).transpose(1, 0, 2, 3)).astype(BF)

    vidx = np.arange(VC * 128).reshape(VC, 128)
    oh = (fea[:, None, None, :] == vidx[None, :, :, None])       # (B, VC, 128, N)
    oh = np.ascontiguousarray(oh.transpose(0, 2, 1, 3)).astype(BF)

    # mnT[g, p, d, c, q] = mneg[c*128+q, d*128+p] ; mneg = -1000*(1-adj)
    mneg = MASK_NEG * (1.0 - (adj > 0))                          # (B, N, N) n,m
    mnT = mneg.transpose(0, 2, 1).reshape(B, NC, 128, NC, 128)   # g, d, p, c, q
    mnT = np.ascontiguousarray(mnT.transpose(0, 2, 1, 3, 4)).astype(BF)
    npm = np.ascontiguousarray(
        non_pad_mask.reshape(B, NC, 128).transpose(0, 2, 1)).astype(np.float32)

    return oh, mnT, npm, embw, e2w, woaug, negw


def kernel(fea, adj, non_pad_mask, embed, W_heads, a_heads, W_out, a_out,
           _trace=False):
    oh, mnT, npm, embw, e2w, woaug, negw = _host_prep(
        fea, adj, non_pad_mask, embed, W_heads, a_heads, W_out, a_out)

    nc = build_kernel()
    identbf = np.eye(128, dtype=np.float32).astype(BF)
    ones3 = np.ones((65, 128), dtype=np.float32).astype(BF)
    in_maps = []
    for i in range(NCORES):
        sl = slice(i * GPC, (i + 1) * GPC)
        in_maps.append({
            "oh": oh[sl], "mnT": mnT[sl], "npm": npm[sl],
            "embw": embw, "e12w": e2w, "woaug": woaug, "negw": negw,
            "identbf": identbf, "ones3": ones3,
        })
    res = run_bass_kernel_spmd(nc, in_maps, core_ids=list(range(NCORES)),
                               trace=_trace)
    outs = []
    for i in range(NCORES):
        o = res.results[i]["out"]                   # (GPC, 128, NC, OUT)
        outs.append(o.transpose(0, 2, 1, 3).reshape(GPC, N, OUT))
    full = np.concatenate(outs, axis=0).astype(np.float32)
    if _trace:
        kernel.last_results = res
    return full
